# revision 54
# baseline (speedup 1.0000x reference)
"""PointFlow2DVAE loss kernel for 8 Trainium2 NeuronCores.

Data-parallel over batch B=8: one point cloud per core. Each core runs the
PointNet encoder, the combined Euler CNF integration (forward + generation
branch) with the exact-divergence computation folded into precomputed
matmuls, and the chamfer distance. Cores emit small partial-sum vectors;
the final scalar combine happens on host.

Scheduling: all heavy phases are emitted as software-pipelined wavefronts
(stage-major across point tiles / branches) so every engine queue always
holds independent work. This removes the dependency stalls of a depth-first
emission and keeps the PE array continuously busy (full p-state).
"""
import numpy as np

import concourse.bacc as bacc
import concourse.bass as bass
import concourse.tile as tile
from concourse import mybir
from concourse.bass_utils import run_bass_kernel_spmd

B, N, D = 8, 2048, 2
LAT, ENC_H, CNF_H = 128, 256, 256
STEPS = 10
DT = 1.0 / STEPS
LAM_R, LAM_P, LAM_E, LAM_C, LAM_V = 1.0, 0.1, 0.01, 10.0, 0.01
LOG2PI = float(np.log(2.0 * np.pi))

NT = 512
NNT = N // NT
NC = STEPS * NNT * 2          # CNF combos: (step, nt, half)
F32 = mybir.dt.float32
BF16 = mybir.dt.bfloat16
MDT = mybir.dt.float32r      # fp32r: full fp32 numerics, 1 cycle/row at ap>=256
FP8 = mybir.dt.float8e4      # e4m3: 0.5 cycles/row in DoubleRow (K=256/instr)
PM = mybir.MatmulPerfMode

AF = mybir.ActivationFunctionType
ALU = mybir.AluOpType
AX = mybir.AxisListType


def host_precompute(w):
    f = np.float32
    W1, b1, W2, b2, W3, b3 = w["W1"], w["b1"], w["W2"], w["b2"], w["W3"], w["b3"]
    pre = {}
    pre["enc1"] = np.ascontiguousarray(w["We1"].T, f)                 # [2,256]
    pre["be1c"] = np.ascontiguousarray(w["be1"].reshape(2, 128).T, f)  # [128,2]
    pre["We2T"] = np.ascontiguousarray(
        w["We2"].T.reshape(2, 128, 256).transpose(1, 0, 2), f)        # [128,2,256]
    pre["be2c"] = np.ascontiguousarray(w["be2"].reshape(2, 128).T, f)  # [128,2]
    pre["WmuT"] = np.ascontiguousarray(
        w["Wmu"].T.reshape(2, 128, 128).transpose(1, 0, 2), f)        # [128,2,128]
    pre["WlvT"] = np.ascontiguousarray(
        w["Wlv"].T.reshape(2, 128, 128).transpose(1, 0, 2), f)
    pre["bmulv"] = np.ascontiguousarray(
        np.stack([w["bmu"], w["blv"]], 1), f)                         # [128,2]

    W1p = W1[:, 0:2]
    W1t = W1[:, 2]
    pre["W1zT"] = np.ascontiguousarray(W1[:, 3:].T, f)                # [128,256]
    # a1top: W1p.T repeated for 20 evals x 2 m-blocks: [2, 20, 2, 128]
    pre["a1top"] = np.ascontiguousarray(
        np.broadcast_to(W1p.T.reshape(2, 1, 2, 128), (2, 20, 2, 128)), f)
    idx = np.arange(STEPS)
    pb3 = W1p @ b3
    TB_e = (idx * DT)[:, None] * W1t[None, :] + b1[None, :] \
        + (idx * DT)[:, None] * pb3[None, :]
    TB_g = (1.0 - idx * DT)[:, None] * W1t[None, :] + b1[None, :] \
        - (idx * DT)[:, None] * pb3[None, :]
    pre["TB"] = np.ascontiguousarray(np.concatenate([TB_e, TB_g], 0), f)  # [20,256]

    pre["W2T"] = np.ascontiguousarray(
        W2.T.reshape(2, 128, 256).transpose(1, 0, 2), f)              # [128,2,256]
    pre["b2c"] = np.ascontiguousarray(b2.reshape(2, 128).T, f)        # [128,2]

    pre["vW"] = np.ascontiguousarray(
        (DT * W3.T).reshape(2, 128, 2).transpose(1, 0, 2), f)         # [128,2,2]
    pre["nvW"] = np.ascontiguousarray(
        (-DT * W3.T).reshape(2, 128, 2).transpose(1, 0, 2), f)

    c0, c1 = W1[:, 0], W1[:, 1]
    Wu = (W3[0][:, None] * W2) * c0[None, :] + (W3[1][:, None] * W2) * c1[None, :]
    pre["nWuT"] = np.ascontiguousarray(
        (-Wu.T).reshape(2, 128, 256).transpose(1, 0, 2), f)           # [128,2,256]
    pre["rur"] = np.ascontiguousarray(Wu.sum(1)[None, :], f)          # [1,256]

    pre["identb"] = np.eye(128, dtype=f)
    pre["b3c"] = np.ascontiguousarray(b3[:, None], f)                 # [2,1]
    pre["nb3c"] = np.ascontiguousarray(-b3[:, None], f)
    return pre


WEIGHT_SPECS = [
    ("enc1", (2, 256)), ("be1c", (128, 2)), ("We2T", (128, 2, 256)),
    ("be2c", (128, 2)),
    ("WmuT", (128, 2, 128)), ("WlvT", (128, 2, 128)), ("bmulv", (128, 2)),
    ("W1zT", (128, 256)), ("a1top", (2, 20, 2, 128)), ("TB", (20, 256)),
    ("W2T", (128, 2, 256)), ("vW", (128, 2, 2)),
    ("nvW", (128, 2, 2)), ("nWuT", (128, 2, 256)),
    ("b3c", (2, 1)), ("nb3c", (2, 1)), ("identb", (128, 128)),
    ("b2c", (128, 2)),
]

# which constants ride which DMA queue at t=0 (keeps every sequencer's
# issue latency off the critical path of the engine that needs it first)
Q_SYNC = ["enc1", "be1c", "We2T", "be2c", "WmuT", "WlvT", "bmulv",
          "TB", "W1zT", "b2c"]
# rest (a1top handled separately, vW/nvW/nWuT/identb/b3c/nb3c) -> gpsimd

# DRAM tensors feeding matmul operands are declared fp32r (identical fp32
# storage) so non-gpsimd queues can DMA them without a dtype cast.
MM_DRAM = {"xT3", "nT3", "enc1", "We2T", "a1top",
           "W2T", "vW", "nvW", "nWuT"}


def build_nc(zero_bias=False):
    nc = bacc.Bacc("TRN2", target_bir_lowering=False, debug=False,
                   enable_asserts=False, num_devices=B)
    nc._zero_bias = zero_bias
    ins = {}
    ins["xT3"] = nc.dram_tensor("xT3", [3, N], MDT, kind="ExternalInput").ap()
    ins["nT3"] = nc.dram_tensor("nT3", [3, N], MDT, kind="ExternalInput").ap()
    ins["epsc"] = nc.dram_tensor("epsc", [LAT, 1], F32, kind="ExternalInput").ap()
    for name, shape in WEIGHT_SPECS:
        dt_ = MDT if name in MM_DRAM else F32
        ins[name] = nc.dram_tensor(name, list(shape), dt_, kind="ExternalInput").ap()
    outs = {}
    for name, shape in [("o_div", [128]), ("o_mu", [128]), ("o_lv", [128]),
                        ("o_chA", [128]), ("o_chB", [128]), ("o_sy2", [2]),
                        ("o_h2s", [128, 2])]:
        outs[name] = nc.dram_tensor(name, shape, F32, kind="ExternalOutput").ap()

    with tile.TileContext(nc) as tc:
        _body(nc, tc, ins, outs)
    nc.compile()
    return nc


def _body(nc, tc, ins, outs):
    from contextlib import ExitStack
    zero_bias = nc._zero_bias
    with ExitStack() as ctx:
        const = ctx.enter_context(tc.tile_pool(name="const", bufs=1))
        state = ctx.enter_context(tc.tile_pool(name="state", bufs=1))
        work = ctx.enter_context(tc.tile_pool(name="work", bufs=2))
        small = ctx.enter_context(tc.tile_pool(name="small", bufs=1))

        # SP issue order: first encoder tile + its weights, then the rest —
        # the encoder's first matmul gates the whole pipeline ramp
        xT3 = state.tile([4, N], MDT, tag="st3", bufs=3, name="xT3")
        nc.sync.dma_start(out=xT3[0:3, 0:NT], in_=ins["xT3"][:, 0:NT])

        # ---- load constants (spread across DMA queues) ----
        c = {}
        order = Q_SYNC + [n for n, _ in WEIGHT_SPECS if n not in Q_SYNC]
        shapes = dict(WEIGHT_SPECS)
        emitted_xt3_rest = False
        for name in order:
            if name == "We2T" and not emitted_xt3_rest:
                for nt in range(1, NNT):
                    sl = slice(nt * NT, (nt + 1) * NT)
                    nc.sync.dma_start(out=xT3[0:3, sl], in_=ins["xT3"][:, sl])
                emitted_xt3_rest = True
            if name == "a1top":
                continue  # DMA'd straight from DRAM into a1w below
            shape = shapes[name]
            if name in ("W2T", "nWuT"):
                dt_ = FP8   # DoubleRow operands; gpsimd DMA casts on load
            elif name == "identb":
                dt_ = BF16
            else:
                dt_ = MDT if name in MM_DRAM else F32
            c[name] = const.tile(list(shape), dt_, tag=name, name=f"c_{name}")
            if name in Q_SYNC:
                nc.sync.dma_start(out=c[name], in_=ins[name])
            else:
                nc.gpsimd.dma_start(out=c[name], in_=ins[name])

        # a1w: [3, 20, 2, 128] K=3 stationary operands (W1p rows + bias row).
        # gpsimd queue, first so it lands before the CNF needs it.
        a1w = state.tile([3, 20, 2, 128], MDT, tag="a1w")
        nc.gpsimd.dma_start(out=a1w[0:2], in_=ins["a1top"])
        # euler/gen state, double-buffered per n-tile: step i reads buf[i%2],
        # writes buf[(i+1)%2]; row 2 is the ones row for the K=3 a1 matmul.
        ybuf = [[], []]
        sbuf_ = [[], []]
        for nt in range(NNT):
            sl = slice(nt * NT, (nt + 1) * NT)
            for p in range(2):
                yt = state.tile([3, NT], MDT, tag=f"y{nt}_{p}", name=f"y{nt}_{p}")
                st_ = state.tile([3, NT], MDT, tag=f"s{nt}_{p}", name=f"s{nt}_{p}")
                if p == 0:
                    nc.sync.dma_start(out=yt, in_=ins["xT3"][:, sl])
                    nc.sync.dma_start(out=st_, in_=ins["nT3"][:, sl])
                ybuf[p].append(yt)
                sbuf_[p].append(st_)
        # ones rows of the write-side (p=1) state buffers, needed ~20us in
        for nt in range(NNT):
            sl = slice(nt * NT, (nt + 1) * NT)
            nc.sync.dma_start(out=ybuf[1][nt][2:3], in_=ins["xT3"][2:3, sl])
            nc.sync.dma_start(out=sbuf_[1][nt][2:3], in_=ins["xT3"][2:3, sl])
        eps_s = small.tile([LAT, 1], F32, tag="eps")
        nc.scalar.dma_start(out=eps_s, in_=ins["epsc"])

        # chamfer aug tiles; the ones rows come straight from DRAM early.
        # Row pairing (dot over k): xneg = [-2x; x^2; 1,1], rplain = [r; 1,1; r^2]
        ones_dram = ins["xT3"][2:3]
        xneg = state.tile([6, N], MDT, tag="st3", bufs=3, name="xneg")
        rplain = state.tile([6, N], MDT, tag="st3", bufs=3, name="rplain")
        nc.gpsimd.dma_start(out=xneg[4:6], in_=ones_dram.partition_broadcast(2))
        nc.gpsimd.dma_start(out=rplain[2:4], in_=ones_dram.partition_broadcast(2))

        divslots = small.tile([128, NC // 2], F32, tag="divslots")
        h2slots = small.tile([128, 2, NC // 2], F32, tag="h2slots")

        # ================= encoder (wavefront over n-tiles) =================
        g_s = small.tile([128, 2], F32, tag="g")
        gparts = small.tile([128, 2, NNT], F32, tag="gparts")
        psCNF = ExitStack()
        psA = psCNF.enter_context(tc.tile_pool(name="psA", bufs=2, space="PSUM"))
        psB = psCNF.enter_context(tc.tile_pool(name="psB", bufs=2, space="PSUM"))

        A1E, H1E, A2E, H2E = {}, {}, {}, {}
        for t in range(NNT + 2):
            if t < NNT:
                sl = slice(t * NT, (t + 1) * NT)
                a1e = psA.tile([128, 2 * NT], F32, tag="a1", name="a1e")
                for mb in range(2):
                    mbs = slice(mb * 128, (mb + 1) * 128)
                    nc.tensor.matmul(a1e[:, mb * NT:(mb + 1) * NT],
                                     c["enc1"][:, mbs], xT3[0:2, sl],
                                     start=True, stop=True)
                h1e = work.tile([128, 2 * NT], MDT, tag="h1", bufs=3, name="h1e")
                if zero_bias:
                    nc.scalar.activation(h1e, a1e, AF.Relu)
                else:
                    for mb in range(2):
                        ms = slice(mb * NT, (mb + 1) * NT)
                        nc.scalar.activation(h1e[:, ms], a1e[:, ms], AF.Relu,
                                             bias=c["be1c"][:, mb:mb + 1])
                A1E[t], H1E[t] = a1e, h1e
            if 1 <= t <= NNT:
                h1e = H1E[t - 1]
                a2e = psB.tile([128, 2 * NT], F32, tag="a2", name="a2e")
                for mb in range(2):
                    mbs = slice(mb * 128, (mb + 1) * 128)
                    om = a2e[:, mb * NT:(mb + 1) * NT]
                    nc.tensor.matmul(om, c["We2T"][:, 0, mbs],
                                     h1e[:, 0:NT], start=True, stop=False)
                    nc.tensor.matmul(om, c["We2T"][:, 1, mbs],
                                     h1e[:, NT:2 * NT], start=False, stop=True)
                h2e = work.tile([128, 2 * NT], MDT, tag="h2", bufs=3, name="h2e")
                if zero_bias:
                    nc.scalar.activation(h2e, a2e, AF.Relu)
                else:
                    for mb in range(2):
                        ms = slice(mb * NT, (mb + 1) * NT)
                        nc.scalar.activation(h2e[:, ms], a2e[:, ms], AF.Relu,
                                             bias=c["be2c"][:, mb:mb + 1])
                A2E[t - 1], H2E[t - 1] = a2e, h2e
            if t >= 2:
                h2e = H2E[t - 2]
                for mb in range(2):
                    nc.vector.tensor_reduce(gparts[:, mb, t - 2:t - 1],
                                            h2e[:, mb * NT:(mb + 1) * NT],
                                            axis=AX.X, op=ALU.max)
        for mb in range(2):
            nc.vector.tensor_reduce(g_s[:, mb:mb + 1], gparts[:, mb, :],
                                    axis=AX.X, op=ALU.max)

        # --- z chain: mu/logvar -> z -> cz row -> bias rows for a1w ---
        mu_ps = psA.tile([128, 1], F32, tag="a1", name="mu_ps")
        lv_ps = psA.tile([128, 1], F32, tag="a1", name="lv_ps")
        for kb in range(2):
            nc.tensor.matmul(mu_ps, c["WmuT"][:, kb, :], g_s[:, kb:kb + 1],
                             start=(kb == 0), stop=(kb == 1))
            nc.tensor.matmul(lv_ps, c["WlvT"][:, kb, :], g_s[:, kb:kb + 1],
                             start=(kb == 0), stop=(kb == 1))
        mu_s = small.tile([128, 1], F32, tag="mu_s")
        lv_s = small.tile([128, 1], F32, tag="lv_s")
        nc.vector.tensor_scalar(mu_s, mu_ps, c["bmulv"][:, 0:1], None, ALU.add)
        nc.vector.tensor_scalar(lv_s, lv_ps, c["bmulv"][:, 1:2], None, ALU.add)
        nc.sync.dma_start(out=outs["o_mu"], in_=mu_s)
        nc.sync.dma_start(out=outs["o_lv"], in_=lv_s)
        # z = mu + eps * exp(0.5*lv)
        e_s = small.tile([128, 1], F32, tag="e_s")
        nc.scalar.activation(e_s, lv_s, AF.Exp, scale=0.5)
        z_s = small.tile([128, 1], F32, tag="z_s")
        nc.vector.tensor_tensor(z_s, e_s, eps_s, ALU.mult)
        nc.vector.tensor_tensor(z_s, z_s, mu_s, ALU.add)
        # cz_row = z @ W1zT : [1, 256]; broadcast to 20 eval rows on-chip
        cz_ps = psB.tile([1, 256], F32, tag="a2", name="cz_ps")
        nc.tensor.matmul(cz_ps, z_s, c["W1zT"], start=True, stop=True)
        czrow = small.tile([1, 256], F32, tag="czrow")
        nc.vector.tensor_copy(czrow, cz_ps)
        czb = work.tile([20, 256], F32, tag="czb", bufs=1, name="czb")
        nc.gpsimd.partition_broadcast(czb, czrow)
        brows = state.tile([20, 256], MDT, tag="brows")
        nc.vector.tensor_tensor(brows, c["TB"], czb, ALU.add)
        nc.sync.dma_start(out=a1w[2:3].rearrange("a b c d -> a (b c d)"),
                          in_=brows)

        # x-side chamfer prep (independent of CNF; fills early DVE slack).
        # DVE cannot shift partitions, so x^2 lands in a [2,N] tile and a
        # small SBUF->SBUF DMA drops it into xneg rows 2:4.
        nc.vector.tensor_scalar(xneg[0:2], xT3[0:2], -2.0, None, ALU.mult)
        sqx = work.tile([2, N], MDT, tag="sqx", bufs=1, name="sqx")
        nc.vector.tensor_tensor(sqx, xT3[0:2], xT3[0:2], ALU.mult)
        nc.scalar.dma_start(out=xneg[2:4], in_=sqx)

        # ================= CNF euler+gen: software-pipelined wavefront ======
        # combo c -> (step, nt, half); half 0 = euler, 1 = gen.
        def cmap(cc):
            i, r = divmod(cc, 2 * NNT)
            nt, half = divmod(r, 2)
            return i, nt, half

        A1, A2, UPS, VPS, H1, H2, H1SQ = {}, {}, {}, {}, {}, {}, {}

        def emit_a1(cc):
            i, nt, half = cmap(cc)
            stl = ybuf if half == 0 else sbuf_
            evi = i if half == 0 else STEPS + i
            st = stl[i % 2][nt]
            a1 = psA.tile([128, 2 * NT], F32, tag="a1", name="a1")
            for mb in range(2):
                nc.tensor.matmul(a1[:, mb * NT:(mb + 1) * NT],
                                 a1w[:, evi, mb, :], st[0:3, :],
                                 start=True, stop=True)
            A1[cc] = a1

        def emit_h1(cc):
            h1 = work.tile([128, 2, NT], FP8, tag="h1f8", bufs=4, name="h1")
            nc.scalar.activation(h1.rearrange("p a b -> p (a b)"), A1[cc],
                                 AF.Tanh)
            H1[cc] = h1

        def emit_a2(cc):
            # fp8 DoubleRow: both K-halves contract in one instruction
            h1 = H1[cc]
            a2 = psB.tile([128, 2 * NT], F32, tag="a2", name="a2")
            for mb in range(2):
                mbs = slice(mb * 128, (mb + 1) * 128)
                nc.tensor.matmul(a2[:, mb * NT:(mb + 1) * NT],
                                 c["W2T"][:, :, mbs], h1,
                                 start=True, stop=True, perf_mode=PM.DoubleRow)
            A2[cc] = a2

        def emit_h1sq(cc):
            h1sq = work.tile([128, 2, NT], FP8, tag="h1sqf8", name="h1sq")
            nc.gpsimd.tensor_tensor(h1sq, H1[cc], H1[cc], ALU.mult)
            H1SQ[cc] = h1sq

        def emit_h2(cc):
            h2 = work.tile([128, 2 * NT], MDT, tag="h2", bufs=3, name="h2")
            if zero_bias:
                nc.scalar.activation(h2, A2[cc], AF.Tanh)
            else:
                for mb in range(2):
                    ms = slice(mb * NT, (mb + 1) * NT)
                    nc.scalar.activation(h2[:, ms], A2[cc][:, ms], AF.Tanh,
                                         bias=c["b2c"][:, mb:mb + 1])
            H2[cc] = h2

        def emit_ups(cc):
            h1sq = H1SQ[cc]
            ups = psA.tile([128, 2 * NT], F32, tag="a1", name="ups")
            for mb in range(2):
                mbs = slice(mb * 128, (mb + 1) * 128)
                nc.tensor.matmul(ups[:, mb * NT:(mb + 1) * NT],
                                 c["nWuT"][:, :, mbs], h1sq,
                                 start=True, stop=True, perf_mode=PM.DoubleRow)
            UPS[cc] = ups

        def emit_vps(cc):
            i, nt, half = cmap(cc)
            vw = c["vW"] if half == 0 else c["nvW"]
            h2 = H2[cc]
            vps = psB.tile([2, NT], F32, tag="a2", name="vps")
            for kb in range(2):
                nc.tensor.matmul(vps, vw[:, kb, :], h2[:, kb * NT:(kb + 1) * NT],
                                 start=(kb == 0), stop=(kb == 1))
            VPS[cc] = vps

        def emit_div(cc):
            # h2sq per mb with rowsum accum; scr = (h2sq - 1) * ups, col-summed
            i, nt, half = cmap(cc)
            slot = i * NNT + nt
            h2 = H2[cc]
            h2sq = work.tile([128, 2 * NT], F32, tag="h2sq", name="h2sq")
            for mb in range(2):
                ms = slice(mb * NT, (mb + 1) * NT)
                nc.vector.scalar_tensor_tensor(
                    out=h2sq[:, ms], in0=h2[:, ms], scalar=1.0,
                    in1=h2[:, ms], op0=ALU.mult, op1=ALU.mult,
                    accum_out=h2slots[:, mb, slot:slot + 1])
            scr = work.tile([128, 2 * NT], F32, tag="scr", name="scr")
            nc.vector.scalar_tensor_tensor(
                out=scr, in0=h2sq, scalar=1.0, in1=UPS[cc],
                op0=ALU.subtract, op1=ALU.mult,
                accum_out=divslots[:, slot:slot + 1])

        def emit_stn(cc):
            i, nt, half = cmap(cc)
            stl = ybuf if half == 0 else sbuf_
            st = stl[i % 2][nt]
            stn = stl[(i + 1) % 2][nt]
            nc.vector.tensor_tensor(stn[0:2, :], st[0:2, :], VPS[cc], ALU.add)

        for cc in range(NC + 2):
            # PE stream
            if cc < NC:
                emit_a1(cc)
            if 1 <= cc <= NC:
                emit_a2(cc - 1)
            if cc >= 2:
                if cmap(cc - 2)[2] == 0:
                    emit_ups(cc - 2)
                emit_vps(cc - 2)
            # Act stream
            if cc < NC:
                emit_h1(cc)
            if 1 <= cc <= NC:
                emit_h2(cc - 1)
            # Pool stream
            if 1 <= cc <= NC and cmap(cc - 1)[2] == 0:
                emit_h1sq(cc - 1)
            # DVE stream
            if cc >= 2:
                if cmap(cc - 2)[2] == 0:
                    emit_div(cc - 2)
                emit_stn(cc - 2)

        psCNF.close()  # release psA/psB banks before the chamfer D pool

        divacc = small.tile([128, 1], F32, tag="divacc")
        nc.vector.tensor_reduce(divacc, divslots, axis=AX.X, op=ALU.add)
        nc.sync.dma_start(out=outs["o_div"], in_=divacc)
        h2rows = small.tile([128, 2], F32, tag="h2rows")
        nc.vector.tensor_reduce(h2rows, h2slots, axis=AX.X, op=ALU.add)
        nc.sync.dma_start(out=outs["o_h2s"], in_=h2rows)

        # final y stats: y_true = y + b3 (per n-tile)
        sy2slots = small.tile([2, NNT], F32, tag="sy2slots")
        for nt in range(NNT):
            ytrue = work.tile([2, NT], F32, tag="yt", name="ytrue")
            nc.vector.tensor_scalar(ytrue, ybuf[STEPS % 2][nt][0:2, :],
                                    c["b3c"], None, ALU.add)
            sy2scr = work.tile([2, NT], F32, tag="scr2", name="sy2scr")
            nc.scalar.activation(sy2scr, ytrue, AF.Square,
                                 accum_out=sy2slots[:, nt:nt + 1])
        sy2 = small.tile([2, 1], F32, tag="sy2")
        nc.vector.tensor_reduce(sy2, sy2slots, axis=AX.X, op=ALU.add)
        nc.sync.dma_start(out=outs["o_sy2"], in_=sy2)

        # ================= chamfer =================
        # Single D computation, K=6 augmented matmul (fp32r PSUM accumulate):
        #   lhsT = rplain[:, blk] = [r_x; r_y; r_x^2; r_y^2; 1; 1]
        #   rhs  = xneg = [-2x_x; -2x_y; 1; 1; x_x^2; x_y^2]
        # Each D block is then copied PSUM->SBUF as bf16 (on the otherwise
        # idle Act/Pool engines) so both min passes run on DVE in fast mode.
        # bf16 rounds the *final* distances (0.4% rel) - harmless here.
        # r-side prep rides the idle Pool engine, per n-tile so the first D
        # blocks can start as soon as tile 0's rows land (DVE is the chamfer
        # bottleneck; keep it off the prep path entirely)
        sqr = work.tile([2, N], MDT, tag="sqr", bufs=1, name="sqr")
        for nt in range(NNT):
            sl = slice(nt * NT, (nt + 1) * NT)
            if zero_bias:  # b3 == 0: r = final gen state as-is
                nc.gpsimd.tensor_copy(rplain[0:2, sl],
                                      sbuf_[STEPS % 2][nt][0:2, :])
            else:  # per-partition scalar needs TensorScalarPtr (DVE-only)
                nc.vector.tensor_scalar(rplain[0:2, sl],
                                        sbuf_[STEPS % 2][nt][0:2, :],
                                        c["nb3c"], None, ALU.add)
            nc.gpsimd.tensor_tensor(sqr[:, sl], rplain[0:2, sl],
                                    rplain[0:2, sl], ALU.mult)
            nc.sync.dma_start(out=rplain[4:6, sl], in_=sqr[:, sl])

        chAmax = small.tile([128, 16], BF16, tag="chAmax")
        # Act negates each D block on the PSUM->SBUF bf16 copy, so both
        # running-"min" chains are TT max (the only order op Pool supports):
        # even blocks fold on DVE, odd blocks on Pool, merged at the end.
        # bf16 SBUF also puts the DVE reduces in fast mode.
        runA = state.tile([128, N], BF16, tag="runA")
        # D computed in [128, 1024] half-blocks (r-block x x-half), 4-deep
        # PSUM rotation: the matmul -> Act-negcopy -> free WAR loop no longer
        # gates the pipeline. The two x-halves run disjoint max chains into
        # runA's column halves (no merge); per-r maxes pair up afterwards.
        chAhalf = small.tile([128, 16, 2], BF16, tag="chAhalf")
        chBmax = small.tile([128, 16], F32, tag="chBmax")
        DSB = {}
        NH = N // 2
        # blk-major with a 4-deep PSUM rotation; per-x running-max chains per
        # x-half on DVE; transposes after the loop.
        with tc.tile_pool(name="psD", bufs=4, space="PSUM") as psD:
            for hb in range(32):
                blk, half = divmod(hb, 2)
                bsl = slice(blk * 128, (blk + 1) * 128)
                Dp = psD.tile([128, NH], F32, tag="D", name="Dp")
                for mt in range(2):
                    msl = slice(half * NH + mt * NT, half * NH + (mt + 1) * NT)
                    nc.tensor.matmul(Dp[:, mt * NT:(mt + 1) * NT],
                                     rplain[:, bsl], xneg[:, msl],
                                     start=True, stop=True)
                Dsb = work.tile([128, NH], BF16, tag="dsb", bufs=8, name="Dsb")
                nc.scalar.mul(Dsb, Dp, -1.0)
                DSB[hb] = Dsb
                nc.vector.tensor_reduce(chAhalf[:, blk, half:half + 1], Dsb,
                                        axis=AX.X, op=ALU.max)
                # Pool's GPSIMD ISA has no order ops, so both running-max
                # chains live on DVE (TT max, bf16 fast mode)
                rsl = slice(0, NH) if half == 0 else slice(NH, N)
                if hb == 2:
                    nc.vector.tensor_tensor(runA[:, rsl], DSB[0], DSB[2],
                                            ALU.max)
                elif hb == 3:
                    nc.vector.tensor_tensor(runA[:, rsl], DSB[1], DSB[3],
                                            ALU.max)
                elif hb >= 4:
                    nc.vector.tensor_tensor(runA[:, rsl], runA[:, rsl], Dsb,
                                            ALU.max)
        with tc.tile_pool(name="psT", bufs=4, space="PSUM") as psT:
            for k in range(16):
                tp = psT.tile([128, 128], BF16, tag="T", name="tp")
                nc.tensor.transpose(tp, runA[:, k * 128:(k + 1) * 128],
                                    c["identb"])
                nc.vector.tensor_reduce(chBmax[:, k:k + 1], tp,
                                        axis=AX.X, op=ALU.max)
        # per-r side: pair the x-half maxes, clamp(-max(-D)), sqrt, row sums
        nc.vector.tensor_tensor(chAmax, chAhalf[:, :, 0], chAhalf[:, :, 1],
                                ALU.max)
        chs = small.tile([128, 16], F32, tag="chs")
        red = small.tile([128, 1], F32, tag="red")
        chsA = small.tile([128, 16], F32, tag="chsA")
        nc.vector.tensor_scalar(chsA, chAmax, -1.0, 0.0, ALU.mult, ALU.max)
        nc.scalar.activation(chs, chsA, AF.Sqrt)
        nc.vector.tensor_reduce(red, chs, axis=AX.X, op=ALU.add)
        nc.sync.dma_start(out=outs["o_chA"], in_=red)
        chsB = small.tile([128, 16], F32, tag="chsB")
        nc.vector.tensor_scalar(chsB, chBmax, -1.0, 0.0, ALU.mult, ALU.max)
        chs2 = small.tile([128, 16], F32, tag="chs2")
        red2 = small.tile([128, 1], F32, tag="red2")
        nc.scalar.activation(chs2, chsB, AF.Sqrt)
        nc.vector.tensor_reduce(red2, chs2, axis=AX.X, op=ALU.add)
        nc.sync.dma_start(out=outs["o_chB"], in_=red2)


_NC_CACHE = {}


def _get_nc(zero_bias=True):
    key = ("nc", zero_bias)
    if key not in _NC_CACHE:
        _NC_CACHE[key] = build_nc(zero_bias=zero_bias)
    return _NC_CACHE[key]


def kernel(**inputs):
    inputs = {k: np.asarray(v, dtype=np.float32) if np.asarray(v).dtype != np.int32
              else np.asarray(v) for k, v in inputs.items()}
    pre = host_precompute(inputs)
    zb = not (np.any(inputs["b2"]) or np.any(inputs["be1"])
              or np.any(inputs["be2"]) or np.any(inputs["b3"]))
    nc = _get_nc(zero_bias=bool(zb))

    ones_row = np.ones((1, N), np.float32)
    in_maps = []
    for b in range(B):
        m = dict(pre)
        m["xT3"] = np.ascontiguousarray(
            np.concatenate([inputs["x"][b].T, ones_row], 0), np.float32)
        m["nT3"] = np.ascontiguousarray(
            np.concatenate([inputs["noise"][b].T, ones_row], 0), np.float32)
        m["epsc"] = np.ascontiguousarray(inputs["eps"][b][:, None], np.float32)
        in_maps.append(m)

    res = run_bass_kernel_spmd(nc, in_maps, core_ids=list(range(B)))
    return combine(res.results, pre)


def combine(results, pre):
    ru2 = pre["rur"].reshape(2, 128).astype(np.float64)
    S_logpy = 0.0
    S_logdet = 0.0
    prior = 0.0
    entropy = 0.0
    chamA = 0.0
    chamB = 0.0
    for r in results:
        S_logpy += -0.5 * float(r["o_sy2"].sum()) - N * LOG2PI
        H = r["o_h2s"].astype(np.float64)  # [128, 2] = (partition, mb)
        corr = float((ru2 * (STEPS * N - H.T)).sum())
        S_logdet += DT * (corr - float(r["o_div"].sum()))
        mu = r["o_mu"].astype(np.float64)
        lv = r["o_lv"].astype(np.float64)
        prior += 0.5 * float((mu ** 2 + np.exp(lv) - lv - 1.0).sum())
        entropy += -0.5 * float((lv + 1.0 + LOG2PI).sum())
        chamA += float(r["o_chA"].sum())
        chamB += float(r["o_chB"].sum())
    recon = -(S_logpy + S_logdet) / (B * N)
    prior /= B
    entropy /= B
    cham = chamA / (B * N) + chamB / (B * N)
    vol = max(0.0, S_logdet / (B * N) - 10.0)
    return np.float32(LAM_R * recon + LAM_P * prior + LAM_E * entropy
                      + LAM_C * cham + LAM_V * vol)
